# revision 1
# baseline (speedup 1.0000x reference)
"""GAE (generalized advantage estimation) Trainium2 kernel.

Problem: nn_CustomGAE — B=512, T=2048, D=64.
  value = obs @ W + b ; next_value = next_obs @ W + b
  td0 = reward + gamma*nd*next_value - value ; coef = gamma*lambda*nd
  A_t = td0_t + coef_t * A_{t+1}  (reverse scan over T, independent per trajectory)
  returns (advantage, value_target = advantage + value)

Sharding: pure data parallel over B across 8 cores (64 trajectories/core).

Per-core layout: the host pre-swizzles each 64-trajectory shard to
(half, batch)-major, so SBUF partition p = h*64 + b holds timesteps
t in [h*1024, (h+1)*1024) at a uniform DRAM stride — every streamed chunk is
one 128-partition dma_start with 16KB-contiguous per-partition runs.

The value-head matvec streams obs/next_obs in chunks: DVE does obs*W
(in-place) + both segmented reduces, GPSIMD does next_obs*W, so no single
engine exceeds the HBM DMA time. The backward recurrence runs as DVE
tensor_tensor_scan over reversed-stride APs (second half first, the
boundary value carried to the first half via a tiny SBUF->SBUF DMA).
"""

import sys

sys.path.insert(0, "/opt/trn_rl_repo")

from contextlib import ExitStack

import numpy as np

import concourse.bacc as bacc
import concourse.mybir as mybir
import concourse.tile as tile
from concourse.bass_utils import run_bass_kernel_spmd

GAMMA = 0.99
LMBDA = 0.95

B, T, D = 512, 2048, 64
NCORES = 8
BL = B // NCORES  # 64 trajectories per core
H = 2  # trajectory halves stacked on partitions -> 128 partitions
P = H * BL  # 128
F32 = mybir.dt.float32
U8 = mybir.dt.uint8

# Results of the last hardware run, for test harnesses.
LAST_RESULTS = None


def _build_iter(
    nc, opool, npool, ppool, dpool, w_t, b_t, bnd,
    obs_d, nobs_d, rw_d, dn_d, adv_d, tgt_d, tp, tc_sz, nchunk,
    dual_dma=False, nocompute=False, out_scalar=True,
):
    """One full pass: load inputs, matvec, scan, write outputs.

    Engine/ring discipline: ALL input streaming runs on the SP (sync) HWDGE
    ring with waits only on pool-slot availability, so it never stalls
    behind compute. Output + boundary DMAs go on the Activation ring.
    Tiles written early in an iteration but read late in the previous one
    (v_raw, nv_raw, rw_t, dn_t) come from a bufs=2 pool so back-to-back
    iterations don't serialize on WAW."""
    mult = mybir.AluOpType.mult
    add = mybir.AluOpType.add
    sub = mybir.AluOpType.subtract
    # second HWDGE engine (Activation) for the next_obs stream
    eng2 = nc.scalar if dual_dma else nc.sync
    oeng = nc.scalar if out_scalar else nc.sync

    v_raw = dpool.tile([P, tp], F32)  # obs @ W (no bias)
    nv_raw = dpool.tile([P, tp], F32)  # next_obs @ W (no bias)
    rw_t = dpool.tile([P, tp], F32)
    dn_t = dpool.tile([P, tp], U8)
    nc.sync.dma_start(rw_t[:], rw_d.ap())
    nc.sync.dma_start(dn_t[:], dn_d.ap())

    # done -> nd-derived factors, issued BEFORE the chunk stream so they
    # overlap it instead of sitting in the post-stream serial tail.
    ndf = dpool.tile([P, tp], F32)
    nc.vector.tensor_copy(ndf[:], dn_t[:])  # u8 -> f32
    g = dpool.tile([P, tp], F32)  # gamma * nd
    nc.scalar.activation(
        g[:], ndf[:], mybir.ActivationFunctionType.Copy, bias=GAMMA, scale=-GAMMA
    )
    coef = dpool.tile([P, tp], F32)  # gamma * lambda * nd
    nc.scalar.activation(
        coef[:],
        ndf[:],
        mybir.ActivationFunctionType.Copy,
        bias=GAMMA * LMBDA,
        scale=-GAMMA * LMBDA,
    )

    wb = w_t[:].unsqueeze(1).broadcast_to([P, tc_sz, D])
    for j in range(nchunk):
        ot = opool.tile([P, tc_sz * D], F32)
        nt = npool.tile([P, tc_sz * D], F32)
        fs = slice(j * tc_sz * D, (j + 1) * tc_sz * D)
        nc.sync.dma_start(ot[:], obs_d.ap()[:, fs])
        eng2.dma_start(nt[:], nobs_d.ap()[:, fs])
        if nocompute:
            continue
        o3 = ot[:].rearrange("p (t d) -> p t d", d=D)
        n3 = nt[:].rearrange("p (t d) -> p t d", d=D)
        cs = slice(j * tc_sz, (j + 1) * tc_sz)
        nc.vector.tensor_tensor(out=o3, in0=o3, in1=wb, op=mult)
        nc.vector.tensor_reduce(
            out=v_raw[:, cs], in_=o3, axis=mybir.AxisListType.X, op=add
        )
        nc.gpsimd.tensor_tensor(out=n3, in0=n3, in1=wb, op=mult)
        nc.vector.tensor_reduce(
            out=nv_raw[:, cs], in_=n3, axis=mybir.AxisListType.X, op=add
        )
    if nocompute:
        # still write outputs so the IO footprint matches (garbage values;
        # rw_t is used because it is actually written by a DMA above)
        oeng.dma_start(adv_d.ap(), rw_t[:])
        oeng.dma_start(tgt_d.ap(), rw_t[:])
        return

    # epilogue: td0 = reward + gamma*nd*(nv_raw+b) - (v_raw+b)
    nvb = ppool.tile([P, tp], F32)
    nc.vector.tensor_scalar_add(nvb[:], nv_raw[:], b_t[:, 0:1])
    vb = ppool.tile([P, tp], F32)  # value = v_raw + b
    nc.vector.tensor_scalar_add(vb[:], v_raw[:], b_t[:, 0:1])
    q = ppool.tile([P, tp], F32)
    nc.vector.tensor_tensor(out=q[:], in0=g[:], in1=nvb[:], op=mult)
    s = ppool.tile([P, tp], F32)
    nc.vector.tensor_tensor(out=s[:], in0=rw_t[:], in1=vb[:], op=sub)
    td0 = ppool.tile([P, tp], F32)
    nc.vector.tensor_tensor(out=td0[:], in0=q[:], in1=s[:], op=add)

    # Backward scan: second half (partitions 64..127, later timesteps)
    # first; its t'=0 element is A at the first half's boundary.
    adv = ppool.tile([P, tp], F32)
    hi = slice(BL, 2 * BL)
    lo = slice(0, BL)
    nc.vector.tensor_tensor_scan(
        out=adv[hi, ::-1],
        data0=coef[hi, ::-1],
        data1=td0[hi, ::-1],
        initial=0.0,
        op0=mult,
        op1=add,
    )
    oeng.dma_start(bnd[:], adv[hi, 0:1])
    nc.vector.tensor_tensor_scan(
        out=adv[lo, ::-1],
        data0=coef[lo, ::-1],
        data1=td0[lo, ::-1],
        initial=bnd[:, 0:1],
        op0=mult,
        op1=add,
    )

    tgt = ppool.tile([P, tp], F32)
    nc.vector.tensor_tensor(out=tgt[:], in0=adv[:], in1=vb[:], op=add)

    oeng.dma_start(adv_d.ap(), adv[:])
    oeng.dma_start(tgt_d.ap(), tgt[:])


def build_program(
    t_total=T, nchunk=16, repeat=1, dual_dma=False, nocompute=False, bufs=3,
    bench_internal=False, out_scalar=True, dbl=2, obufs=4,
):
    """Build the per-core Bass program (all 8 cores run it SPMD on their own
    shard). DRAM tensor layouts are (half, batch)-major as produced by
    shard_inputs. repeat>1 re-runs the whole pipeline inside one NEFF
    (test.py uses the delta vs repeat=1 to measure per-iteration HW time).
    bench_internal makes obs/next_obs Internal DRAM (not shipped per call;
    garbage values) so benchmark calls are cheap — timing-only builds."""
    tp = t_total // H  # timesteps per partition
    tc_sz = tp // nchunk  # timesteps per streamed chunk
    assert tp % nchunk == 0

    nc = bacc.Bacc(
        "TRN2", target_bir_lowering=False, debug=False, enable_asserts=False
    )

    big_kind = "Internal" if bench_internal else "ExternalInput"
    obs_d = nc.dram_tensor("obs", [P, tp * D], F32, kind=big_kind)
    nobs_d = nc.dram_tensor("nobs", [P, tp * D], F32, kind=big_kind)
    rw_d = nc.dram_tensor("rw", [P, tp], F32, kind="ExternalInput")
    dn_d = nc.dram_tensor("dn", [P, tp], U8, kind="ExternalInput")
    w_d = nc.dram_tensor("w", [D], F32, kind="ExternalInput")
    b_d = nc.dram_tensor("b", [1], F32, kind="ExternalInput")
    adv_d = nc.dram_tensor("adv", [P, tp], F32, kind="ExternalOutput")
    tgt_d = nc.dram_tensor("tgt", [P, tp], F32, kind="ExternalOutput")

    with tile.TileContext(nc) as tc, ExitStack() as ctx:
        cpool = ctx.enter_context(tc.tile_pool(name="const", bufs=1))
        opool = ctx.enter_context(tc.tile_pool(name="obs", bufs=obufs or bufs))
        npool = ctx.enter_context(tc.tile_pool(name="nobs", bufs=bufs))
        ppool = ctx.enter_context(tc.tile_pool(name="pers", bufs=1))
        dpool = ctx.enter_context(tc.tile_pool(name="dbl", bufs=dbl))

        # Value-head weights replicated to every partition.
        w_t = cpool.tile([P, D], F32)
        nc.sync.dma_start(w_t[:], w_d.ap().unsqueeze(0).broadcast_to([P, D]))
        b_t = cpool.tile([P, 1], F32)
        nc.sync.dma_start(b_t[:], b_d.ap().unsqueeze(0).broadcast_to([P, 1]))

        bnd = cpool.tile([BL, 1], F32)

        for _rep in range(repeat):
            _build_iter(
                nc, opool, npool, ppool, dpool, w_t, b_t, bnd,
                obs_d, nobs_d, rw_d, dn_d, adv_d, tgt_d, tp, tc_sz, nchunk,
                dual_dma=dual_dma, nocompute=nocompute, out_scalar=out_scalar,
            )

    # Runs the bacc pipeline (register allocation etc.) — required before
    # serializing for the walrus compiler.
    nc.finalize()
    return nc


_NC_CACHE = None


def _get_nc():
    global _NC_CACHE
    if _NC_CACHE is None:
        _NC_CACHE = build_program()
    return _NC_CACHE


def _hmajor(x, tp_cols):
    """[BL, H*tp_cols] row-major -> [H*BL, tp_cols] with row p = h*BL + b."""
    return np.ascontiguousarray(
        x.reshape(BL, H, tp_cols).transpose(1, 0, 2).reshape(H * BL, tp_cols)
    )


def _unhmajor(y):
    """Inverse of _hmajor for outputs: [H*BL, tp] -> [BL, H*tp]."""
    tp = y.shape[1]
    return y.reshape(H, BL, tp).transpose(1, 0, 2).reshape(BL, H * tp)


def shard_inputs(obs, next_obs, reward, done, W, b):
    """Split full inputs into the 8 per-core input maps ((h,b)-major)."""
    obs = np.asarray(obs, dtype=np.float32).reshape(B, T * D)
    nobs = np.asarray(next_obs, dtype=np.float32).reshape(B, T * D)
    rw = np.asarray(reward, dtype=np.float32).reshape(B, T)
    dn = np.asarray(done).astype(np.uint8, copy=False).reshape(B, T)
    w_np = np.ascontiguousarray(np.asarray(W, dtype=np.float32)).reshape(D)
    b_np = np.ascontiguousarray(np.asarray(b, dtype=np.float32)).reshape(1)

    tpd = (T // H) * D
    tp = T // H
    in_maps = []
    for i in range(NCORES):
        sl = slice(i * BL, (i + 1) * BL)
        in_maps.append(
            {
                "obs": _hmajor(obs[sl], tpd),
                "nobs": _hmajor(nobs[sl], tpd),
                "rw": _hmajor(rw[sl], tp),
                "dn": _hmajor(dn[sl], tp),
                "w": w_np,
                "b": b_np,
            }
        )
    return in_maps


def gather_outputs(results):
    advantage = np.concatenate(
        [_unhmajor(r["adv"]) for r in results], axis=0
    ).reshape(B, T, 1)
    value_target = np.concatenate(
        [_unhmajor(r["tgt"]) for r in results], axis=0
    ).reshape(B, T, 1)
    return advantage, value_target


def kernel(obs, next_obs, reward, done, W, b):
    global LAST_RESULTS
    nc = _get_nc()
    in_maps = shard_inputs(obs, next_obs, reward, done, W, b)
    res = run_bass_kernel_spmd(nc, in_maps, core_ids=list(range(NCORES)))
    LAST_RESULTS = res
    return gather_outputs(res.results)

